# revision 1
# baseline (speedup 1.0000x reference)
"""FPN ROI-Align pooler (nn_Pooler) as a Bass/Tile kernel on 8 Trainium2 cores.

Design (v2):
  - Host builds a 2-row-banded channels-last fp16 table: entry (img,y,x) holds
    feature rows y and y+1 for pixel column x -> [106250+pad, 2*256].
  - Each output bin needs a KxW window x 2 y-samples: one indirect-DMA offset
    per (bin, y-sample) fetches K consecutive band entries (K*2*256 contig).
    Partition = (bin, y-half): 64 bins per gather instruction.
  - Bilinear + sample-average reduce to 2K weighted terms per partition, done
    on PE as 2K block-diagonal matmuls accumulating in PSUM [64,256] f32.
    DVE builds the block-diag weight tiles (broadcast x mask), ACT copies
    PSUM->SBUF, HWDGE stores.
  - Bins are classified by x-window K in {4,8,16}, dealt round-robin to the 8
    cores so every core runs the identical SPMD program.
"""

import sys

import numpy as np

if "/opt/trn_rl_repo" not in sys.path:
    sys.path.insert(0, "/opt/trn_rl_repo")

OUT = 7
SR = 2
SCALES = (0.25, 0.125, 0.0625, 0.03125)
K_MIN = 2
CANON_SCALE = 224.0
CANON_LVL = 4.0
EPS = 1e-6

B, C, N = 2, 256, 1000
SIZES = ((200, 200), (100, 100), (50, 50), (25, 25))
NCORES = 8
NBIN = OUT * OUT
# class config: (K, noff) = x-window width in band entries, offsets per bin.
# noff=1: y-merged single fetch; noff=2: per-y-sample (or far-ym per-x-sample)
# fetches; noff=4: far bins, one 2-col fetch per (y-sample, x-sample).
CFG = ((2, 1), (2, 2), (2, 4), (3, 1), (3, 2), (4, 1), (4, 2))
CLS_BY = {cfg: i for i, cfg in enumerate(CFG)}
MAXNOFF = 4
MAXNT = 8  # max terms per offset (2K with K<=4)

LEVEL_BASE = []
_acc = 0
for _h, _w in SIZES:
    LEVEL_BASE.append(_acc)
    _acc += B * _h * _w
TOTAL_ROWS = _acc  # 106250
PAD_ROWS = 32
TBL_ROWS = TOTAL_ROWS + PAD_ROWS

_PROGRAM_CACHE: dict = {}


def _axis_precompute(lo, hi, Wdim):
    """Per-axis samples: corners [N,14,2] i32, weights [N,14,2] f32, valid."""
    f32 = np.float32
    roi = np.maximum(hi - lo, f32(1.0))
    bin_sz = roi / f32(OUT)
    a_out = np.arange(OUT, dtype=f32)[None, :, None]
    a_sr = np.arange(SR, dtype=f32)[None, None, :]
    grid = a_out * bin_sz[:, None, None] + (a_sr + f32(0.5)) * bin_sz[:, None, None] / f32(SR)
    pos = (lo[:, None, None] + grid).reshape(N, OUT * SR)
    Wf = Wdim.astype(f32)
    valid = (pos >= f32(-1.0)) & (pos <= Wf[:, None])
    p = np.clip(pos, f32(0.0), (Wf - f32(1.0))[:, None])
    p0f = np.floor(p)
    p0 = p0f.astype(np.int32)
    p1 = np.minimum(p0 + 1, Wdim[:, None] - 1)
    lp = p - p0f
    hp = f32(1.0) - lp
    corn = np.stack([p0, p1], axis=-1)
    wgt = np.stack([hp, lp], axis=-1).astype(f32)
    return corn, wgt, valid


def _host_precompute(boxes, img_ids):
    """Returns idx [N,7,7,2] i32 band-entry offsets, wc [N,7,7,2,16,2] f32,
    kclass [N,7] (per (n,bx) window class index), using banded-table rows."""
    f32 = np.float32
    boxes = np.asarray(boxes, f32)
    x1, y1, x2, y2 = boxes[:, 0], boxes[:, 1], boxes[:, 2], boxes[:, 3]
    area = (x2 - x1) * (y2 - y1)
    s = np.sqrt(area)
    lvl = np.floor(f32(CANON_LVL) + np.log2(s / f32(CANON_SCALE) + f32(EPS)))
    lvl = np.clip(lvl, K_MIN, K_MIN + len(SCALES) - 1).astype(np.int32) - K_MIN

    scale = np.asarray(SCALES, f32)[lvl]
    Hs = np.asarray([h for h, w in SIZES], np.int32)[lvl]
    Ws = np.asarray([w for h, w in SIZES], np.int32)[lvl]
    base = np.asarray(LEVEL_BASE, np.int64)[lvl]

    xcorn, xw, xval = _axis_precompute(x1 * scale, x2 * scale, Ws)
    ycorn, yw, yval = _axis_precompute(y1 * scale, y2 * scale, Hs)

    # --- x side: per (n, bx): window start, per-pixel weights, class
    ix = (np.arange(OUT)[:, None] * SR + np.arange(SR)[None, :])  # [7,2]
    x0a = xcorn[:, ix[:, 0], 0]  # [N,7] window start (first sample lo corner)
    # d offsets for the 4 x-corners
    dcorn = xcorn[:, ix, :] - x0a[:, :, None, None]  # [N,7,2kx,2cx]
    assert dcorn.min() >= 0
    dmax = dcorn.max(axis=(2, 3))  # [N,7]
    assert dmax.max() <= 15, f"x window overflow: {dmax.max()}"
    kclass = np.zeros((N, OUT), np.int32)  # K=2
    kclass[dmax > 1] = 1  # K=3
    kclass[dmax > 2] = 2  # K=4
    kclass[dmax > 3] = 3  # far (per-sample 2-col fetches)

    wx_pix = np.zeros((N, OUT, 16), f32)
    wxc = (xw[:, ix, :] * xval[:, ix][:, :, :, None]) * f32(0.5)  # [N,7,2,2]
    n_i, b_i = np.meshgrid(np.arange(N), np.arange(OUT), indexing="ij")
    for kx in range(SR):
        for cx in range(2):
            np.add.at(wx_pix, (n_i, b_i, dcorn[:, :, kx, cx]), wxc[:, :, kx, cx])

    # --- y side: per (n, by, s): band entry row and 2 row-weights
    iy = (np.arange(OUT)[:, None] * SR + np.arange(SR)[None, :])  # [7,2]
    ybase = ycorn[:, iy, 0]  # [N,7,2]
    wyr = (yw[:, iy, :] * yval[:, iy][:, :, :, None]) * f32(0.5)  # [N,7,2s,2r]

    img = np.asarray(img_ids).astype(np.int64)
    rowy = (img[:, None, None] * Hs[:, None, None].astype(np.int64)
            + ybase.astype(np.int64))  # [N,7,2]
    idx = (base[:, None, None, None]
           + rowy[:, :, None, :] * Ws[:, None, None, None].astype(np.int64)
           + x0a.astype(np.int64)[:, None, :, None])  # [N,by,bx,s]
    assert idx.min() >= 0 and idx.max() < TOTAL_ROWS
    # far bins: per-x-sample window starts
    x0s = xcorn[:, ix, 0]  # [N,7,2xs] per-sample lo corner
    idx4 = (base[:, None, None, None, None]
            + rowy[:, :, None, :, None] * Ws[:, None, None, None, None].astype(np.int64)
            + x0s.astype(np.int64)[:, None, :, None, :])  # [N,by,bx,s,xs]

    ym = idx[..., 0] == idx[..., 1]  # [N,7,7]
    # canonical per-bin packing: cls, idxP [4], wcP [4, MAXNT] (j = d*2+r)
    Nb = N * NBIN
    cls = np.zeros((N, OUT, OUT), np.int32)
    idxP = np.zeros((N, OUT, OUT, MAXNOFF), np.int64)
    wcP = np.zeros((N, OUT, OUT, MAXNOFF, MAXNT), f32)

    wc = (wyr[:, :, None, :, None, :] * wx_pix[:, None, :, None, :, None])
    # wc: [n, by, bx, s, d, r]
    wc_m = wc.sum(axis=3)  # [n,by,bx,d,r] y-merged
    kc3 = np.repeat(kclass[:, None, :], OUT, axis=1)  # [n,by,bx] 0..3 (K2,K3,K4,far)
    # near classes
    for ki, Kv in ((0, 2), (1, 3), (2, 4)):
        m1 = (kc3 == ki) & ym
        m2 = (kc3 == ki) & ~ym
        cls[m1] = CLS_BY[(Kv, 1)]
        cls[m2] = CLS_BY[(Kv, 2)]
        nT = 2 * Kv
        wnear = wc[..., :Kv, :].reshape(N, OUT, OUT, 2, nT)
        wmerg = wc_m[..., :Kv, :].reshape(N, OUT, OUT, nT)
        idxP[m2, :2] = idx[m2]
        wcP[m2, 0, :nT] = wnear[m2][:, 0]
        wcP[m2, 1, :nT] = wnear[m2][:, 1]
        idxP[m1, 0] = idx[m1][:, 0]
        wcP[m1, 0, :nT] = wmerg[m1]
    # far classes: per (s, xs) 2-col fetches, terms j = d*2+r, d in {0,1}
    wxh = wxc  # [N,7(bx),2xs,2cx] per-sample x weights (incl valid*0.5)
    wfar = (wyr[:, :, None, :, None, None, :]
            * wxh[:, None, :, None, :, :, None])  # [n,by,bx,s,xs,d,r]
    wfar = wfar.reshape(N, OUT, OUT, 2, 2, 4)  # [.., s, xs, j]
    mfar = kc3 == 3
    mf2 = mfar & ym   # far-ym: noff=2, one fetch per xs
    mf4 = mfar & ~ym  # far-split: noff=4, fetch per (s, xs)
    cls[mf4] = CLS_BY[(2, 4)]
    cls[mf2] = CLS_BY[(2, 2)]
    idxP[mf4] = idx4[mf4].reshape(-1, 4)
    wcP[mf4, :, :4] = wfar[mf4].reshape(-1, 4, 4)
    idxP[mf2, :2] = idx4[mf2][:, 0]  # rows equal for both s
    wcP[mf2, :2, :4] = wfar[mf2].sum(axis=1)  # sum over s
    return cls.reshape(Nb), idxP.reshape(Nb, MAXNOFF).astype(np.int32), \
        np.ascontiguousarray(wcP.reshape(Nb, MAXNOFF, MAXNT), dtype=f32)


def _make_table(feats, dtype):
    """2-row-banded channels-last table [TBL_ROWS, 2*C]."""
    parts = []
    for f in feats:
        _, _, H, W = f.shape
        nhwc = np.ascontiguousarray(f.transpose(0, 2, 3, 1))  # [B,H,W,C]
        padded = np.concatenate([nhwc, np.zeros((B, 1, W, C), f.dtype)], axis=1)
        band = np.stack([padded[:, :H], padded[:, 1:H + 1]], axis=3)  # [B,H,W,2,C]
        parts.append(band.reshape(-1, 2 * C))
    parts.append(np.zeros((PAD_ROWS, 2 * C), parts[0].dtype))
    return np.ascontiguousarray(np.concatenate(parts, axis=0)).astype(dtype)


def _tile_bins(ci):
    return 128 // CFG[ci][1]


def _plan(cls):
    """Deal bins to cores per class. Returns per-class per-core bin-id grids."""
    plans = []
    for ci in range(len(CFG)):
        ts = _tile_bins(ci)
        ids = np.where(cls == ci)[0]
        padded = -np.ones((int(np.ceil(len(ids) / (ts * NCORES))) * ts * NCORES,),
                          np.int64)
        padded[:len(ids)] = ids
        plans.append(padded.reshape(-1, ts))  # [Tg, ts]
    tcounts = tuple(p.shape[0] // NCORES for p in plans)
    return plans, tcounts


def _pack_core(core, plans, tcounts, idxP, wcP):
    """Per-core DRAM inputs: idx_arr [128,T] i32, wc_arr [128, sum(nT*Tk)] f16,
    slotmap [sum(ts*Tk)] bin ids."""
    T = sum(tcounts)
    idx_arr = np.zeros((128, T), np.int32)
    wc_cols = sum(2 * CFG[ci][0] * tcounts[ci] for ci in range(len(CFG)))
    wc_arr = np.zeros((128, wc_cols), np.float16)
    out_rows = sum(_tile_bins(ci) * tcounts[ci] for ci in range(len(CFG)))
    slotmap = np.full((out_rows,), -1, np.int64)

    t_off = 0
    c_off = 0
    r_off = 0
    for ci, (K, noff) in enumerate(CFG):
        Tk = tcounts[ci]
        nT = 2 * K
        if Tk == 0:
            continue
        ts = _tile_bins(ci)
        tiles = plans[ci][core::NCORES]  # [Tk, ts]
        valid = tiles >= 0
        ids = np.where(valid, tiles, 0)
        iv = idxP[ids][:, :, :noff] * valid[:, :, None]  # [Tk, ts, noff]
        idx_arr[:, t_off:t_off + Tk] = iv.reshape(Tk, 128).T
        wv = wcP[ids][:, :, :noff, :nT] * valid[:, :, None, None]  # [Tk,ts,noff,nT]
        wv = wv.reshape(Tk, 128, nT).transpose(1, 0, 2).reshape(128, Tk * nT)
        wc_arr[:, c_off:c_off + Tk * nT] = wv.astype(np.float16)
        slotmap[r_off:r_off + Tk * ts] = np.where(valid, tiles, -1).reshape(-1)
        t_off += Tk
        c_off += Tk * nT
        r_off += Tk * ts
    return idx_arr, wc_arr, slotmap


def _build_program(tcounts, table_dt_name):
    import concourse.bacc as bacc
    import concourse.bass as bass
    import concourse.tile as tile
    import concourse.mybir as mybir
    from contextlib import ExitStack

    tdt = getattr(mybir.dt, table_dt_name)
    T = sum(tcounts)
    wc_cols = sum(2 * CFG[ci][0] * tcounts[ci] for ci in range(len(CFG)))
    out_rows = sum(_tile_bins(ci) * tcounts[ci] for ci in range(len(CFG)))

    nc = bacc.Bacc("TRN2", target_bir_lowering=False, debug=False)
    tbl = nc.dram_tensor("tbl", [TBL_ROWS, 2 * C], tdt, kind="ExternalInput").ap()
    idxd = nc.dram_tensor("idx", [128, T], mybir.dt.int32, kind="ExternalInput").ap()
    wcd = nc.dram_tensor("wc", [128, wc_cols], mybir.dt.float16, kind="ExternalInput").ap()
    # mask[noff]: [128, MAXNT, 128//noff], pattern delta(b == p//noff)
    maskd = {}
    for noff in (1, 2, 4):
        maskd[noff] = nc.dram_tensor(f"mask{noff}", [128, MAXNT, 128 // noff],
                                     mybir.dt.float16, kind="ExternalInput").ap()
    outd = nc.dram_tensor("out", [out_rows, C], mybir.dt.float16,
                          kind="ExternalOutput").ap()

    with tile.TileContext(nc) as tc, ExitStack() as ctx:
        cpool = ctx.enter_context(tc.tile_pool(name="const", bufs=1))
        gpool = ctx.enter_context(tc.tile_pool(name="g", bufs=6))
        wpool = ctx.enter_context(tc.tile_pool(name="w", bufs=6))
        ppool = ctx.enter_context(tc.tile_pool(name="ps", bufs=6, space="PSUM"))
        opool = ctx.enter_context(tc.tile_pool(name="o", bufs=6))

        idx_sb = cpool.tile([128, T], mybir.dt.int32)
        nc.sync.dma_start(idx_sb[:], idxd[:])
        wc_sb = cpool.tile([128, wc_cols], mybir.dt.float16)
        nc.sync.dma_start(wc_sb[:], wcd[:])
        mask_sb = {}
        for noff in (1, 2, 4):
            mask_sb[noff] = cpool.tile([128, MAXNT, 128 // noff], mybir.dt.float16,
                                       name=f"msk{noff}", tag=f"m{noff}")
            nc.sync.dma_start(mask_sb[noff][:, :, :], maskd[noff][:, :, :])

        # per-class base offsets
        t_offs, c_offs, r_offs = [], [], []
        to = co = ro = 0
        for ci in range(len(CFG)):
            t_offs.append(to); c_offs.append(co); r_offs.append(ro)
            to += tcounts[ci]
            co += tcounts[ci] * 2 * CFG[ci][0]
            ro += tcounts[ci] * _tile_bins(ci)
        # interleave emission round-robin across classes
        order = []
        mx = max(tcounts) if tcounts else 0
        for tt_ in range(mx):
            for ci_ in range(len(CFG)):
                if tt_ < tcounts[ci_]:
                    order.append((ci_, tt_))
        for ci, tt in order:
            K, noff = CFG[ci]
            nT = 2 * K
            ts = _tile_bins(ci)
            msk = mask_sb[noff]
            t_off, c_off, r_off = t_offs[ci], c_offs[ci], r_offs[ci]
            if True:
                t = t_off + tt
                g = gpool.tile([128, nT, C], tdt, tag=f"g{K}")
                nc.gpsimd.indirect_dma_start(
                    out=g[:, :, :].rearrange("p t c -> p (t c)"),
                    out_offset=None,
                    in_=tbl[:],
                    in_offset=bass.IndirectOffsetOnAxis(
                        ap=idx_sb[:, t:t + 1], axis=0),
                )
                wall = wpool.tile([128, nT, ts], mybir.dt.float16, tag=f"wall{ts}")
                co = c_off + tt * nT
                nc.vector.tensor_tensor(
                    out=wall[:, :, :],
                    in0=wc_sb[:, co:co + nT].to_broadcast([128, nT, ts]),
                    in1=msk[:, :nT, :],
                    op=mybir.AluOpType.mult,
                )
                psum = ppool.tile([ts, C], mybir.dt.float32, tag="ps")
                for j in range(nT):
                    nc.tensor.matmul(
                        psum[:],
                        lhsT=wall[:, j, :],
                        rhs=g[:, j, :],
                        start=(j == 0),
                        stop=(j == nT - 1),
                    )
                ob = opool.tile([ts, C], mybir.dt.float16, tag=f"ob{ts}")
                nc.scalar.copy(ob[:], psum[:])
                nc.sync.dma_start(outd[r_off + tt * ts:r_off + (tt + 1) * ts, :], ob[:])

    nc.compile()
    return nc


def _make_mask():
    masks = {}
    p = np.arange(128)
    for noff in (1, 2, 4):
        m = np.zeros((128, MAXNT, 128 // noff), np.float16)
        for j in range(MAXNT):
            m[p, j, p // noff] = 1.0
        masks[noff] = m
    return masks


def prepare(feat0, feat1, feat2, feat3, boxes, img_ids, table_dt="float16"):
    np_dt = np.float16 if table_dt == "float16" else np.float32
    tbl = _make_table((feat0, feat1, feat2, feat3), np_dt)
    cls, idxP, wcP = _host_precompute(boxes, img_ids)
    plans, tcounts = _plan(cls)

    sig = (tcounts, table_dt)
    if sig not in _PROGRAM_CACHE:
        _PROGRAM_CACHE[sig] = _build_program(tcounts, table_dt)
    nc = _PROGRAM_CACHE[sig]

    masks = _make_mask()
    in_maps = []
    slotmaps = []
    for c in range(NCORES):
        idx_arr, wc_arr, slotmap = _pack_core(c, plans, tcounts, idxP, wcP)
        im = {"tbl": tbl, "idx": idx_arr, "wc": wc_arr}
        for noff in (1, 2, 4):
            im[f"mask{noff}"] = masks[noff]
        in_maps.append(im)
        slotmaps.append(slotmap)
    return nc, in_maps, slotmaps


def assemble(results, slotmaps):
    final = np.zeros((N, C, NBIN), np.float32)
    for c in range(NCORES):
        out = results[c]["out"]
        sm = slotmaps[c]
        valid = sm >= 0
        ids = sm[valid]
        final[ids // NBIN, :, ids % NBIN] = out[valid].astype(np.float32)
    return final.reshape(N, C, OUT, OUT)


def kernel(feat0, feat1, feat2, feat3, boxes, img_ids):
    from concourse.bass_utils import run_bass_kernel_spmd

    nc, in_maps, slotmaps = prepare(feat0, feat1, feat2, feat3, boxes, img_ids)
    res = run_bass_kernel_spmd(nc, in_maps, list(range(NCORES)))
    return assemble(res.results, slotmaps)



# revision 3
# speedup vs baseline: 1.0095x; 1.0095x over previous
"""FPN ROI-Align pooler (nn_Pooler) as a Bass/Tile kernel on 8 Trainium2 cores.

Design (v3):
  - Host builds a 2-row-banded channels-last fp8(e3m4) table: entry (img,y,x)
    holds feature rows y and y+1 for pixel column x -> [106250+pad, 2*256].
  - Each output bin needs a KxW window x 2 y-samples: one indirect-DMA offset
    per (bin, y-sample) fetches K consecutive band entries (K*2*256 contig).
    Partition = (bin, y-half): 64 bins per gather tile. Gathers for up to 8
    tiles are merged into one indirect DMA (amortizes SWDGE fixed cost).
  - Bilinear + sample-average reduce to 2K weighted terms per partition, done
    on PE as 2K block-diagonal matmuls. noff tiles share one PSUM [128,C] f32
    via matmul tile_position, so ACT copies + output stores run at full
    128-partition width. Stores are batched 8 slabs per dma_start.
  - DVE builds the block-diag weight tiles as [128, ts, nT] (packed last dim
    for 2x/4x DVE modes) from an fp16 weight row broadcast against a 0/1 mask.
  - Weights stay fp16 (lhsT); gathered data is fp8e3m4 (rhs): mixed-dtype
    matmul. fp8e4m3 fails the 2e-2 precision gate; e3m4 lands at ~1.35e-2.
"""

import sys

import numpy as np

if "/opt/trn_rl_repo" not in sys.path:
    sys.path.insert(0, "/opt/trn_rl_repo")

OUT = 7
SR = 2
SCALES = (0.25, 0.125, 0.0625, 0.03125)
K_MIN = 2
CANON_SCALE = 224.0
CANON_LVL = 4.0
EPS = 1e-6

B, C, N = 2, 256, 1000
SIZES = ((200, 200), (100, 100), (50, 50), (25, 25))
NCORES = 8
NBIN = OUT * OUT
# class config: (K, noff) = x-window width in band entries, offsets per bin.
CFG = ((2, 1), (2, 2), (2, 4), (3, 1), (3, 2), (4, 1), (4, 2))
CLS_BY = {cfg: i for i, cfg in enumerate(CFG)}
MAXNOFF = 4
MAXNT = 8  # max terms per offset (2K with K<=4)
MG = 8     # tiles merged per indirect gather (multiple of every noff)
SLAB_G = 8  # psum-group slabs per output store

LEVEL_BASE = []
_acc = 0
for _h, _w in SIZES:
    LEVEL_BASE.append(_acc)
    _acc += B * _h * _w
TOTAL_ROWS = _acc  # 106250
PAD_ROWS = 32
TBL_ROWS = TOTAL_ROWS + PAD_ROWS

_PROGRAM_CACHE: dict = {}


def _axis_precompute(lo, hi, Wdim):
    """Per-axis samples: corners [N,14,2] i32, weights [N,14,2] f32, valid."""
    f32 = np.float32
    roi = np.maximum(hi - lo, f32(1.0))
    bin_sz = roi / f32(OUT)
    a_out = np.arange(OUT, dtype=f32)[None, :, None]
    a_sr = np.arange(SR, dtype=f32)[None, None, :]
    grid = a_out * bin_sz[:, None, None] + (a_sr + f32(0.5)) * bin_sz[:, None, None] / f32(SR)
    pos = (lo[:, None, None] + grid).reshape(N, OUT * SR)
    Wf = Wdim.astype(f32)
    valid = (pos >= f32(-1.0)) & (pos <= Wf[:, None])
    p = np.clip(pos, f32(0.0), (Wf - f32(1.0))[:, None])
    p0f = np.floor(p)
    p0 = p0f.astype(np.int32)
    p1 = np.minimum(p0 + 1, Wdim[:, None] - 1)
    lp = p - p0f
    hp = f32(1.0) - lp
    corn = np.stack([p0, p1], axis=-1)
    wgt = np.stack([hp, lp], axis=-1).astype(f32)
    return corn, wgt, valid


def _host_precompute(boxes, img_ids):
    """Returns idx [N,7,7,2] i32 band-entry offsets, wc [N,7,7,2,16,2] f32,
    kclass [N,7] (per (n,bx) window class index), using banded-table rows."""
    f32 = np.float32
    boxes = np.asarray(boxes, f32)
    x1, y1, x2, y2 = boxes[:, 0], boxes[:, 1], boxes[:, 2], boxes[:, 3]
    area = (x2 - x1) * (y2 - y1)
    s = np.sqrt(area)
    lvl = np.floor(f32(CANON_LVL) + np.log2(s / f32(CANON_SCALE) + f32(EPS)))
    lvl = np.clip(lvl, K_MIN, K_MIN + len(SCALES) - 1).astype(np.int32) - K_MIN

    scale = np.asarray(SCALES, f32)[lvl]
    Hs = np.asarray([h for h, w in SIZES], np.int32)[lvl]
    Ws = np.asarray([w for h, w in SIZES], np.int32)[lvl]
    base = np.asarray(LEVEL_BASE, np.int64)[lvl]

    xcorn, xw, xval = _axis_precompute(x1 * scale, x2 * scale, Ws)
    ycorn, yw, yval = _axis_precompute(y1 * scale, y2 * scale, Hs)

    # --- x side: per (n, bx): window start, per-pixel weights, class
    ix = (np.arange(OUT)[:, None] * SR + np.arange(SR)[None, :])  # [7,2]
    x0a = xcorn[:, ix[:, 0], 0]  # [N,7] window start (first sample lo corner)
    # d offsets for the 4 x-corners
    dcorn = xcorn[:, ix, :] - x0a[:, :, None, None]  # [N,7,2kx,2cx]
    assert dcorn.min() >= 0
    dmax = dcorn.max(axis=(2, 3))  # [N,7]
    assert dmax.max() <= 15, f"x window overflow: {dmax.max()}"
    kclass = np.zeros((N, OUT), np.int32)  # K=2
    kclass[dmax > 1] = 1  # K=3
    kclass[dmax > 2] = 2  # K=4
    kclass[dmax > 3] = 3  # far (per-sample 2-col fetches)

    wx_pix = np.zeros((N, OUT, 16), f32)
    wxc = (xw[:, ix, :] * xval[:, ix][:, :, :, None]) * f32(0.5)  # [N,7,2,2]
    n_i, b_i = np.meshgrid(np.arange(N), np.arange(OUT), indexing="ij")
    for kx in range(SR):
        for cx in range(2):
            np.add.at(wx_pix, (n_i, b_i, dcorn[:, :, kx, cx]), wxc[:, :, kx, cx])

    # --- y side: per (n, by, s): band entry row and 2 row-weights
    iy = (np.arange(OUT)[:, None] * SR + np.arange(SR)[None, :])  # [7,2]
    ybase = ycorn[:, iy, 0]  # [N,7,2]
    wyr = (yw[:, iy, :] * yval[:, iy][:, :, :, None]) * f32(0.5)  # [N,7,2s,2r]

    img = np.asarray(img_ids).astype(np.int64)
    rowy = (img[:, None, None] * Hs[:, None, None].astype(np.int64)
            + ybase.astype(np.int64))  # [N,7,2]
    idx = (base[:, None, None, None]
           + rowy[:, :, None, :] * Ws[:, None, None, None].astype(np.int64)
           + x0a.astype(np.int64)[:, None, :, None])  # [N,by,bx,s]
    assert idx.min() >= 0 and idx.max() < TOTAL_ROWS
    # far bins: per-x-sample window starts
    x0s = xcorn[:, ix, 0]  # [N,7,2xs] per-sample lo corner
    idx4 = (base[:, None, None, None, None]
            + rowy[:, :, None, :, None] * Ws[:, None, None, None, None].astype(np.int64)
            + x0s.astype(np.int64)[:, None, :, None, :])  # [N,by,bx,s,xs]

    ym = idx[..., 0] == idx[..., 1]  # [N,7,7]
    # canonical per-bin packing: cls, idxP [4], wcP [4, MAXNT] (j = d*2+r)
    Nb = N * NBIN
    cls = np.zeros((N, OUT, OUT), np.int32)
    idxP = np.zeros((N, OUT, OUT, MAXNOFF), np.int64)
    wcP = np.zeros((N, OUT, OUT, MAXNOFF, MAXNT), f32)

    wc = (wyr[:, :, None, :, None, :] * wx_pix[:, None, :, None, :, None])
    # wc: [n, by, bx, s, d, r]
    wc_m = wc.sum(axis=3)  # [n,by,bx,d,r] y-merged
    kc3 = np.repeat(kclass[:, None, :], OUT, axis=1)  # [n,by,bx] 0..3 (K2,K3,K4,far)
    # near classes
    for ki, Kv in ((0, 2), (1, 3), (2, 4)):
        m1 = (kc3 == ki) & ym
        m2 = (kc3 == ki) & ~ym
        cls[m1] = CLS_BY[(Kv, 1)]
        cls[m2] = CLS_BY[(Kv, 2)]
        nT = 2 * Kv
        wnear = wc[..., :Kv, :].reshape(N, OUT, OUT, 2, nT)
        wmerg = wc_m[..., :Kv, :].reshape(N, OUT, OUT, nT)
        idxP[m2, :2] = idx[m2]
        wcP[m2, 0, :nT] = wnear[m2][:, 0]
        wcP[m2, 1, :nT] = wnear[m2][:, 1]
        idxP[m1, 0] = idx[m1][:, 0]
        wcP[m1, 0, :nT] = wmerg[m1]
    # far classes: per (s, xs) 2-col fetches, terms j = d*2+r, d in {0,1}
    wxh = wxc  # [N,7(bx),2xs,2cx] per-sample x weights (incl valid*0.5)
    wfar = (wyr[:, :, None, :, None, None, :]
            * wxh[:, None, :, None, :, :, None])  # [n,by,bx,s,xs,d,r]
    wfar = wfar.reshape(N, OUT, OUT, 2, 2, 4)  # [.., s, xs, j]
    mfar = kc3 == 3
    mf2 = mfar & ym   # far-ym: noff=2, one fetch per xs
    mf4 = mfar & ~ym  # far-split: noff=4, fetch per (s, xs)
    cls[mf4] = CLS_BY[(2, 4)]
    cls[mf2] = CLS_BY[(2, 2)]
    idxP[mf4] = idx4[mf4].reshape(-1, 4)
    wcP[mf4, :, :4] = wfar[mf4].reshape(-1, 4, 4)
    idxP[mf2, :2] = idx4[mf2][:, 0]  # rows equal for both s
    wcP[mf2, :2, :4] = wfar[mf2].sum(axis=1)  # sum over s
    return cls.reshape(Nb), idxP.reshape(Nb, MAXNOFF).astype(np.int32), \
        np.ascontiguousarray(wcP.reshape(Nb, MAXNOFF, MAXNT), dtype=f32)


def _make_table(feats, dtype):
    """2-row-banded channels-last table [TBL_ROWS, 2*C]."""
    parts = []
    for f in feats:
        _, _, H, W = f.shape
        nhwc = np.ascontiguousarray(f.transpose(0, 2, 3, 1))  # [B,H,W,C]
        padded = np.concatenate([nhwc, np.zeros((B, 1, W, C), f.dtype)], axis=1)
        band = np.stack([padded[:, :H], padded[:, 1:H + 1]], axis=3)  # [B,H,W,2,C]
        parts.append(band.reshape(-1, 2 * C))
    parts.append(np.zeros((PAD_ROWS, 2 * C), parts[0].dtype))
    return np.ascontiguousarray(np.concatenate(parts, axis=0)).astype(dtype)


def _tile_bins(ci):
    return 128 // CFG[ci][1]


def _plan(cls):
    """Deal bins to cores per class. Returns per-class per-core bin-id grids.

    Per-core tile count for class ci is padded to a multiple of noff so that
    noff tiles always share one [128, C] PSUM."""
    plans = []
    for ci in range(len(CFG)):
        ts = _tile_bins(ci)
        noff = CFG[ci][1]
        chunk = ts * noff * NCORES  # = 128 * NCORES bins
        ids = np.where(cls == ci)[0]
        padded = -np.ones((int(np.ceil(len(ids) / chunk)) * chunk,), np.int64)
        padded[:len(ids)] = ids
        plans.append(padded.reshape(-1, ts))  # [Tg, ts]
    tcounts = tuple(p.shape[0] // NCORES for p in plans)
    return plans, tcounts


def _n_slabs(tcounts):
    return sum(tcounts[ci] // CFG[ci][1] for ci in range(len(CFG)))


def _pack_core(core, plans, tcounts, idxP, wcP):
    """Per-core DRAM inputs: idx_arr [128,T] i32, wc_arr [128, sum(nT*Tk)] f16,
    slotmap [n_slabs*128] bin ids (row = slab*128 + psum_partition)."""
    T = sum(tcounts)
    idx_arr = np.zeros((128, T), np.int32)
    wc_cols = sum(2 * CFG[ci][0] * tcounts[ci] for ci in range(len(CFG)))
    wc_arr = np.zeros((128, wc_cols), np.float16)
    nslab = _n_slabs(tcounts)
    slotmap = np.full((nslab, 128), -1, np.int64)

    t_off = 0
    c_off = 0
    s_off = 0
    for ci, (K, noff) in enumerate(CFG):
        Tk = tcounts[ci]
        nT = 2 * K
        if Tk == 0:
            continue
        ts = _tile_bins(ci)
        tiles = plans[ci][core::NCORES]  # [Tk, ts]
        valid = tiles >= 0
        ids = np.where(valid, tiles, 0)
        iv = idxP[ids][:, :, :noff] * valid[:, :, None]  # [Tk, ts, noff]
        idx_arr[:, t_off:t_off + Tk] = iv.reshape(Tk, 128).T
        wv = wcP[ids][:, :, :noff, :nT] * valid[:, :, None, None]  # [Tk,ts,noff,nT]
        wv = wv.reshape(Tk, 128, nT).transpose(1, 0, 2).reshape(128, Tk * nT)
        wc_arr[:, c_off:c_off + Tk * nT] = wv.astype(np.float16)
        # slab layout: psum group g covers tiles (g*noff .. g*noff+noff-1);
        # psum partition p = jj*ts + b -> bin tiles[g*noff+jj, b]
        ng = Tk // noff
        sm = np.where(valid, tiles, -1).reshape(ng, noff * ts)  # [ng, 128]
        slotmap[s_off:s_off + ng] = sm
        t_off += Tk
        c_off += Tk * nT
        s_off += ng
    return idx_arr, wc_arr, slotmap.reshape(-1)


def _build_program(tcounts, table_dt_name):
    import concourse.bacc as bacc
    import concourse.bass as bass
    import concourse.tile as tile
    import concourse.mybir as mybir
    from contextlib import ExitStack

    tdt = getattr(mybir.dt, table_dt_name)
    T = sum(tcounts)
    wc_cols = sum(2 * CFG[ci][0] * tcounts[ci] for ci in range(len(CFG)))
    nslab = _n_slabs(tcounts)

    nc = bacc.Bacc("TRN2", target_bir_lowering=False, debug=False)
    tbl = nc.dram_tensor("tbl", [TBL_ROWS, 2 * C], tdt, kind="ExternalInput").ap()
    idxd = nc.dram_tensor("idx", [128, T], mybir.dt.int32, kind="ExternalInput").ap()
    wcd = nc.dram_tensor("wc", [128, wc_cols], mybir.dt.float16, kind="ExternalInput").ap()
    # mask[noff]: [128, ts, MAXNT], value delta(b == p//noff), j-independent
    maskd = {}
    for noff in (1, 2, 4):
        maskd[noff] = nc.dram_tensor(f"mask{noff}", [128, 128 // noff, MAXNT],
                                     mybir.dt.float16, kind="ExternalInput").ap()
    outd = nc.dram_tensor("out", [128, nslab, C], mybir.dt.float16,
                          kind="ExternalOutput").ap()

    with tile.TileContext(nc) as tc, ExitStack() as ctx:
        cpool = ctx.enter_context(tc.tile_pool(name="const", bufs=1))
        gpool = ctx.enter_context(tc.tile_pool(name="g", bufs=2))
        wpool = ctx.enter_context(tc.tile_pool(name="w", bufs=6))
        ppool = ctx.enter_context(tc.tile_pool(name="ps", bufs=6, space="PSUM"))
        opool = ctx.enter_context(tc.tile_pool(name="o", bufs=2))

        idx_sb = cpool.tile([128, T], mybir.dt.int32)
        nc.sync.dma_start(idx_sb[:], idxd[:])
        wc_sb = cpool.tile([128, wc_cols], mybir.dt.float16)
        nc.sync.dma_start(wc_sb[:], wcd[:])
        mask_sb = {}
        for noff in (1, 2, 4):
            mask_sb[noff] = cpool.tile([128, 128 // noff, MAXNT], mybir.dt.float16,
                                       name=f"msk{noff}", tag=f"m{noff}")
            nc.sync.dma_start(mask_sb[noff][:, :, :], maskd[noff][:, :, :])

        # per-class base offsets
        t_offs, c_offs, s_offs = [], [], []
        to = co = so = 0
        for ci in range(len(CFG)):
            t_offs.append(to); c_offs.append(co); s_offs.append(so)
            to += tcounts[ci]
            co += tcounts[ci] * 2 * CFG[ci][0]
            so += tcounts[ci] // CFG[ci][1]

        # gather events (ci, t0, G), interleaved round-robin across classes
        ev_by_class = []
        for ci in range(len(CFG)):
            evs = []
            for t0 in range(0, tcounts[ci], MG):
                evs.append((ci, t0, min(MG, tcounts[ci] - t0)))
            ev_by_class.append(evs)
        order = []
        mx = max((len(e) for e in ev_by_class), default=0)
        for i in range(mx):
            for evs in ev_by_class:
                if i < len(evs):
                    order.append(evs[i])

        # output staging: slabs accumulate in [128, SLAB_G, C]; slab index is
        # global across classes in emission order of psum groups
        stage = {"tile": None, "base": 0, "fill": 0}

        def flush_stage():
            if stage["tile"] is not None and stage["fill"] > 0:
                gsz = stage["fill"]
                nc.sync.dma_start(
                    outd[:, stage["base"]:stage["base"] + gsz, :],
                    stage["tile"][:, :gsz, :])
            stage["tile"] = None

        slab_seq = [0]  # global slab counter in emission order

        for ci, t0, G in order:
            K, noff = CFG[ci]
            nT = 2 * K
            ts = _tile_bins(ci)
            msk = mask_sb[noff]
            t_off, c_off = t_offs[ci], c_offs[ci]
            g = gpool.tile([128, G, nT, C], tdt, tag=f"g{K}n{noff}g{G}")
            nc.gpsimd.indirect_dma_start(
                out=g[:, :, :, :].rearrange("p g t c -> p (g t c)"),
                out_offset=None,
                in_=tbl[:],
                in_offset=bass.IndirectOffsetOnAxis(
                    ap=idx_sb[:, t_off + t0:t_off + t0 + G], axis=0),
            )
            for gg in range(0, G, noff):
                psum = ppool.tile([128, C], mybir.dt.float32, tag="ps")
                for jj in range(noff):
                    tt = t0 + gg + jj
                    wall = wpool.tile([128, ts, nT], mybir.dt.float16,
                                      tag=f"wall{ts}x{nT}")
                    co = c_off + tt * nT
                    wrow = wc_sb[:, co:co + nT]
                    w_bc = bass.AP(wrow.tensor, wrow.offset,
                                   [list(wrow.ap[0]), [0, ts], list(wrow.ap[1])])
                    nc.vector.tensor_tensor(
                        out=wall[:, :, :],
                        in0=w_bc,
                        in1=msk[:, :, :nT],
                        op=mybir.AluOpType.mult,
                    )
                    for j in range(nT):
                        nc.tensor.matmul(
                            psum[jj * ts:(jj + 1) * ts, :],
                            lhsT=wall[:, :, j],
                            rhs=g[:, gg + jj, j, :],
                            start=(j == 0),
                            stop=(j == nT - 1),
                            tile_position=(0, jj * ts),
                        )
                if stage["tile"] is None:
                    stage["tile"] = opool.tile([128, SLAB_G, C], mybir.dt.float16,
                                               tag="stage")
                    stage["base"] = slab_seq[0]
                    stage["fill"] = 0
                nc.scalar.copy(stage["tile"][:, stage["fill"], :], psum[:, :])
                stage["fill"] += 1
                slab_seq[0] += 1
                if stage["fill"] == SLAB_G:
                    flush_stage()
        flush_stage()

    nc.compile()
    return nc


def _make_mask():
    masks = {}
    p = np.arange(128)
    for noff in (1, 2, 4):
        m = np.zeros((128, 128 // noff, MAXNT), np.float16)
        m[p, p // noff, :] = 1.0
        masks[noff] = m
    return masks


def prepare(feat0, feat1, feat2, feat3, boxes, img_ids, table_dt="float8e3"):
    if table_dt == "float8e3":
        import ml_dtypes
        np_dt = ml_dtypes.float8_e3m4
    elif table_dt == "float16":
        np_dt = np.float16
    else:
        np_dt = np.float32
    tbl = _make_table((feat0, feat1, feat2, feat3), np_dt)
    cls, idxP, wcP = _host_precompute(boxes, img_ids)
    plans, tcounts = _plan(cls)

    sig = (tcounts, table_dt)
    if sig not in _PROGRAM_CACHE:
        _PROGRAM_CACHE[sig] = _build_program(tcounts, table_dt)
    nc = _PROGRAM_CACHE[sig]

    masks = _make_mask()
    in_maps = []
    slotmaps = []
    for c in range(NCORES):
        idx_arr, wc_arr, slotmap = _pack_core(c, plans, tcounts, idxP, wcP)
        im = {"tbl": tbl, "idx": idx_arr, "wc": wc_arr}
        for noff in (1, 2, 4):
            im[f"mask{noff}"] = masks[noff]
        in_maps.append(im)
        slotmaps.append(slotmap)
    return nc, in_maps, slotmaps


def _emission_slab_order(tcounts):
    """Map emission-order slab index -> packing-order slab index.

    _pack_core lays slabs out class-major; _build_program emits psum groups
    in gather-event order (round-robin over classes at MG-tile granularity).
    Returns perm with perm[emit_idx] = pack_idx."""
    s_offs = []
    so = 0
    for ci in range(len(CFG)):
        s_offs.append(so)
        so += tcounts[ci] // CFG[ci][1]
    ev = []
    for ci in range(len(CFG)):
        for t0 in range(0, tcounts[ci], MG):
            ev.append((ci, t0, min(MG, tcounts[ci] - t0)))
    # round-robin interleave, same as _build_program
    by_class = {}
    for ci, t0, G in ev:
        by_class.setdefault(ci, []).append((ci, t0, G))
    order = []
    mx = max((len(v) for v in by_class.values()), default=0)
    for i in range(mx):
        for ci in range(len(CFG)):
            evs = by_class.get(ci, [])
            if i < len(evs):
                order.append(evs[i])
    perm = []
    for ci, t0, G in order:
        noff = CFG[ci][1]
        for gg in range(0, G, noff):
            perm.append(s_offs[ci] + (t0 + gg) // noff)
    return np.asarray(perm, np.int64)


def assemble(results, slotmaps):
    # tcounts implied by slotmap length is identical across cores
    final = np.zeros((N, C, NBIN), np.float32)
    for c in range(NCORES):
        out = results[c]["out"]  # [128, nslab, C]
        rows = np.ascontiguousarray(out.transpose(1, 0, 2)).reshape(-1, C)
        sm = slotmaps[c]
        valid = sm >= 0
        ids = sm[valid]
        final[ids // NBIN, :, ids % NBIN] = rows[valid].astype(np.float32)
    return final.reshape(N, C, OUT, OUT)


def kernel(feat0, feat1, feat2, feat3, boxes, img_ids):
    from concourse.bass_utils import run_bass_kernel_spmd

    nc, in_maps, slotmaps = prepare(feat0, feat1, feat2, feat3, boxes, img_ids)
    res = run_bass_kernel_spmd(nc, in_maps, list(range(NCORES)))
    return assemble(res.results, slotmaps)
